# revision 41
# baseline (speedup 1.0000x reference)
"""Paged GQA chunked-prefill attention for 8 Trainium2 NeuronCores.

Problem (hardcoded): B=4 seqs x Q=256 new tokens, H=32 query heads, KVH=8 kv
heads (GQA group G=4), D=128 head dim, paged KV cache of 512 blocks x 16
tokens, per-seq lengths in seq_lens (clamped to >= Q), causal masking.

Sharding: tensor-parallel over heads. Core h gets kv head h and query heads
h*4..h*4+3; block_tables/seq_lens are resolved host-side while packing the
shards; the output is all-gathered host-side over the hidden dim.

Per-core device kernel, all matmul operands fp16 (full PE rate, fp32 PSUM
accumulation). For seq b, kv chunk c of 128 positions, q = (t,g) -> 1024
columns in two 512-column halves n; fully-masked column prefixes of boundary
chunks are clipped out of every stage:
  S^T[kv,q] = K_c^T q                 (fp16 matmul into PSUM)
  S^T += causal mask                  (identity-lhsT matmul, boundary band)
  U = exp(SCALE * S^T)                (one ScalarE pass over the active cols)
  lb[128,q] += ones128^T @ U          (all-ones lhsT: the denominator lands
                                       broadcast across all 128 partitions,
                                       so the epilogue needs no cross-
                                       partition moves at all)
  O^T[d,q] += V_c^T @ U               (PSUM accumulation over chunks)
Runs of unmasked chunks accumulate their U on the VectorE (fp16 adds) so one
half-width l matmul covers the whole run. Scores/exp are emitted three chunks
ahead of l/PV consumption (software pipeline) so neither the ScalarE exp nor
the per-sequence epilogues stall the PE. Sequences run longest-first; each
half's epilogue (rl = 1/lb fast-reciprocal straight out of PSUM, then
out = O^T * rl -> fp16, DMA out) is emitted as soon as that half's
accumulation stops, one chunk before the sequence ends for half 0. The PE is
kept warm from t=0 with matmuls on a memset tile (HAM stays at 2.4 GHz) and
the exp table preloads while the input DMAs stream on both HW-DGE rings.
"""
import math

import numpy as np

import concourse.mybir as mybir
import concourse.tile as tile
from concourse import bacc
from concourse.bass_utils import run_bass_kernel_spmd

B, Q, H, D = 4, 256, 32, 128
KVH = 8
G = H // KVH
BLOCK = 16
NB = 128
KV = NB * BLOCK
NUM_BLOCKS = B * NB
SCALE = 1.0 / math.sqrt(D)
N_CORES = 8
CHUNK = 128
QCOLS = G * Q  # 1024 q columns per sequence per core
NHALF = 512

F32 = mybir.dt.float32
F16 = mybir.dt.float16
NEG = -60000.0  # exactly representable in fp16; SCALE*NEG ~ -5300 -> exp = 0


def _plan(seq_lens):
    """Chunk counts, processing order, and tight boundary mask tiles."""
    L = np.maximum(np.asarray(seq_lens, dtype=np.int64), Q)
    cb = [int((int(x) + CHUNK - 1) // CHUNK) for x in L]
    offs = np.concatenate([[0], np.cumsum(cb)]).astype(int)
    border = sorted(range(B), key=lambda b: -cb[b])  # longest first
    # (b, c, n) -> (tmin, tup, mask[128, tup-tmin]); cols t < tmin are fully
    # masked (clipped everywhere), t >= tup fully visible.
    masked = {}
    p = np.arange(CHUNK)
    for b in range(B):
        for c in range(cb[b]):
            for n in range(2):
                lo = int(L[b]) - Q + n * CHUNK  # kv pos of this half's t=0
                if c * CHUNK > lo + CHUNK - 1:
                    continue  # fully masked half
                if c * CHUNK + CHUNK - 1 > lo:
                    tmin = max(0, min(CHUNK, c * CHUNK - lo))
                    tup = max(0, min(CHUNK, c * CHUNK + CHUNK - lo))
                    t = np.arange(tmin, tup)
                    kvpos = c * CHUNK + p
                    m = np.where(
                        kvpos[:, None] > lo + t[None, :], NEG, 0.0
                    ).astype(np.float16)
                    masked[(b, c, n)] = (tmin, tup, m)
    order = sorted(
        masked.keys(), key=lambda k: (border.index(k[0]), k[1], k[2])
    )
    return L, cb, offs, border, masked, order


def _build(seq_lens):
    L, cb, offs, border, masked, order = _plan(seq_lens)
    C = int(offs[-1])
    mask_np = np.concatenate(
        [masked[k][2] for k in order] or [np.zeros((CHUNK, 1), np.float16)],
        axis=1,
    )
    mcols = mask_np.shape[1]
    moff = {}
    acc = 0
    for k in order:
        moff[k] = acc
        acc += masked[k][2].shape[1]
    ident_np = np.eye(CHUNK, dtype=np.float16)
    ones_np = np.ones((CHUNK, CHUNK), dtype=np.float16)

    nc = bacc.Bacc(
        "TRN2", target_bir_lowering=False, debug=False, num_devices=N_CORES
    )
    kt_d = nc.dram_tensor("kt", [D, C * CHUNK], F16, kind="ExternalInput")
    v_d = nc.dram_tensor("v", [CHUNK, C * CHUNK], F16, kind="ExternalInput")
    qt_d = nc.dram_tensor("qt", [D, B * QCOLS], F16, kind="ExternalInput")
    out_d = nc.dram_tensor("out", [B, D, QCOLS], F16, kind="ExternalOutput")
    mask_d = nc.inline_tensor(mask_np, name="mask_const")
    ident_d = nc.inline_tensor(ident_np, name="ident_const")
    ones_d = nc.inline_tensor(ones_np, name="ones_const")

    exp = mybir.ActivationFunctionType.Exp

    def half_lo(b, n):
        return int(L[b]) - Q + n * CHUNK

    def half_state(b, c, n):
        if c * CHUNK > half_lo(b, n) + CHUNK - 1:
            return "skip"
        if (b, c, n) in masked:
            return "mask"
        return "clear"

    def last_chunk(b, n):
        return min(cb[b] - 1, (half_lo(b, n) + CHUNK - 1) // CHUNK)

    def clip_a(b, c, n):
        """First active column (of 512) for this chunk-half."""
        if (b, c, n) in masked:
            return G * masked[(b, c, n)][0]
        return 0

    with tile.TileContext(nc) as tc:
        with (
            tc.tile_pool(name="sbin", bufs=1) as sbin,
            tc.tile_pool(name="sbu", bufs=8) as sbu,
            tc.tile_pool(name="sbp", bufs=2) as sbp,
            tc.tile_pool(name="sbe", bufs=3) as sbe,
            tc.tile_pool(name="ps_s", bufs=2, space="PSUM") as ps_s,
            tc.tile_pool(name="ps_o", bufs=1, space="PSUM") as ps_o,
            tc.tile_pool(name="ps_l", bufs=1, space="PSUM") as ps_l,
        ):
            # Warm the PE + load the exp table before any DMA lands: matmuls
            # on a memset tile keep the HAM activity window busy so the clock
            # is at 2.4 GHz when real data arrives.
            warm_w = sbin.tile([CHUNK, CHUNK], F16, tag="warmw")
            nc.vector.memset(warm_w[:], 0.0)
            dummy = sbe.tile([CHUNK, 1], F32, tag="dummy")
            nc.scalar.activation(dummy[:], warm_w[:, 0:1], exp)
            # l broadcast accumulators (one tile per half so the early
            # half-0 epilogue never false-shares with half-1 matmuls); the
            # first tile doubles as the warmup target -- the first real
            # l matmul (start=True) resets it.
            l_bc0 = ps_l.tile([CHUNK, NHALF], F32, tag="lbc0")
            l_bc1 = ps_l.tile([CHUNK, NHALF], F32, tag="lbc1")
            l_bc = (l_bc0, l_bc1)
            for _ in range(40):
                nc.tensor.matmul(
                    l_bc0[:, 0:CHUNK], warm_w[:], warm_w[:],
                    start=True, stop=True, skip_group_check=True,
                )

            # Input DMAs in processing order (longest seq first). First-seq
            # loads split across the two HW-DGE rings (SP + ACT) so their
            # issue costs don't serialize; head chunks land first so compute
            # can start while the rest of the long sequence streams.
            b0 = border[0]
            kt_t = [None] * B
            v_t = [None] * B
            qt_t = [None] * B
            w0 = cb[b0] * CHUNK
            o0_ = offs[b0] * CHUNK
            head0 = min(4 * CHUNK, w0)
            qt0 = sbin.tile([D, QCOLS], F16, tag=f"qt{b0}")
            nc.sync.dma_start(
                qt0[:, 0:NHALF], qt_d.ap()[:, b0 * QCOLS : b0 * QCOLS + NHALF]
            )
            nc.scalar.dma_start(
                qt0[:, NHALF:QCOLS],
                qt_d.ap()[:, b0 * QCOLS + NHALF : (b0 + 1) * QCOLS],
            )
            kt0 = sbin.tile([D, w0], F16, tag=f"kt{b0}")
            nc.sync.dma_start(kt0[:, 0:head0], kt_d.ap()[:, o0_ : o0_ + head0])
            v0 = sbin.tile([CHUNK, w0], F16, tag=f"v{b0}")
            nc.scalar.dma_start(v0[:, 0:head0], v_d.ap()[:, o0_ : o0_ + head0])
            ones_t = sbin.tile([CHUNK, CHUNK], F16, tag="ones")
            nc.sync.dma_start(ones_t[:], ones_d.ap())
            identr = sbin.tile([CHUNK, CHUNK], F16, tag="identr")
            nc.scalar.dma_start(identr[:], ident_d.ap())
            if head0 < w0:
                mid0 = (head0 + w0 + CHUNK) // (2 * CHUNK) * CHUNK
                nc.sync.dma_start(
                    kt0[:, head0:mid0], kt_d.ap()[:, o0_ + head0 : o0_ + mid0]
                )
                nc.scalar.dma_start(
                    v0[:, head0:mid0], v_d.ap()[:, o0_ + head0 : o0_ + mid0]
                )
                if mid0 < w0:
                    nc.sync.dma_start(
                        kt0[:, mid0:w0], kt_d.ap()[:, o0_ + mid0 : o0_ + w0]
                    )
                    nc.scalar.dma_start(
                        v0[:, mid0:w0], v_d.ap()[:, o0_ + mid0 : o0_ + w0]
                    )
            kt_t[b0], v_t[b0], qt_t[b0] = kt0, v0, qt0

            masks = sbin.tile([CHUNK, mcols], F16, tag="masks")
            cut = sum(
                masked[k][2].shape[1]
                for k in order
                if border.index(k[0]) <= 1
            )
            cut = max(1, min(cut, mcols))
            nc.sync.dma_start(masks[:, 0:cut], mask_d.ap()[:, 0:cut])

            for b in border[1:]:
                w = cb[b] * CHUNK
                o0 = offs[b] * CHUNK
                qt = sbin.tile([D, QCOLS], F16, tag=f"qt{b}")
                nc.sync.dma_start(qt[:], qt_d.ap()[:, b * QCOLS : (b + 1) * QCOLS])
                kt = sbin.tile([D, w], F16, tag=f"kt{b}")
                vt = sbin.tile([CHUNK, w], F16, tag=f"v{b}")
                head = min(4 * CHUNK, w)
                nc.sync.dma_start(kt[:, 0:head], kt_d.ap()[:, o0 : o0 + head])
                nc.sync.dma_start(vt[:, 0:head], v_d.ap()[:, o0 : o0 + head])
                if head < w:
                    nc.sync.dma_start(
                        kt[:, head:w], kt_d.ap()[:, o0 + head : o0 + w]
                    )
                    nc.sync.dma_start(
                        vt[:, head:w], v_d.ap()[:, o0 + head : o0 + w]
                    )
                kt_t[b], v_t[b], qt_t[b] = kt, vt, qt
            if cut < mcols:
                nc.sync.dma_start(
                    masks[:, cut:mcols], mask_d.ap()[:, cut:mcols]
                )

            o_ps0 = ps_o.tile([D, NHALF], F32, tag="o0")
            o_ps1 = ps_o.tile([D, NHALF], F32, tag="o1")
            o_ps = (o_ps0, o_ps1)

            def emit_score(b, c):
                states = [half_state(b, c, n) for n in range(2)]
                s_ps = ps_s.tile([CHUNK, QCOLS], F32, tag="s")
                for n in range(2):
                    if states[n] == "skip":
                        continue
                    h0 = n * NHALF
                    if states[n] == "clear":
                        nc.tensor.matmul(
                            s_ps[:, h0 : h0 + NHALF],
                            kt_t[b][:, c * CHUNK : (c + 1) * CHUNK],
                            qt_t[b][:, h0 : h0 + NHALF],
                            start=True,
                            stop=True,
                        )
                        continue
                    tmin, tup, _ = masked[(b, c, n)]
                    a, u_c = G * tmin, G * tup
                    nc.tensor.matmul(
                        s_ps[:, h0 + a : h0 + u_c],
                        kt_t[b][:, c * CHUNK : (c + 1) * CHUNK],
                        qt_t[b][:, h0 + a : h0 + u_c],
                        start=True,
                        stop=False,
                    )
                    mo = moff[(b, c, n)]
                    mb = (
                        masks[:, mo : mo + (tup - tmin)]
                        .unsqueeze(2)
                        .broadcast_to([CHUNK, tup - tmin, G])
                    )
                    nc.tensor.matmul(
                        s_ps[:, h0 + a : h0 + u_c],
                        identr[:],
                        mb,
                        start=False,
                        stop=True,
                    )
                    if u_c < NHALF:
                        nc.tensor.matmul(
                            s_ps[:, h0 + u_c : h0 + NHALF],
                            kt_t[b][:, c * CHUNK : (c + 1) * CHUNK],
                            qt_t[b][:, h0 + u_c : h0 + NHALF],
                            start=True,
                            stop=True,
                        )
                act0 = (
                    clip_a(b, c, 0)
                    if states[0] != "skip"
                    else NHALF + clip_a(b, c, 1)
                )
                u = sbu.tile([CHUNK, QCOLS], F16, tag="u")
                nc.scalar.activation(
                    u[:, act0:QCOLS], s_ps[:, act0:QCOLS], exp, scale=SCALE
                )
                return u, states

            # (b, n) -> [c_start, acc_ap_or_None, first_u, count]: running
            # U-sum of a clear-chunk run; one l matmul per run.
            lrun = {}

            def emit_l(b, n, rhs_ap, c0_, c1_, last_n, a=0):
                nc.tensor.matmul(
                    l_bc[n][:, a:NHALF],
                    ones_t[:],
                    rhs_ap,
                    start=c0_ == 0,
                    stop=c1_ == last_n,
                    skip_group_check=True,
                )

            def flush_run(b, n, last_n):
                r = lrun.pop((b, n), None)
                if r is None:
                    return
                c0_, acc, fu, k = r
                rhs = fu[:, n * NHALF : (n + 1) * NHALF] if k == 1 else acc[:]
                emit_l(b, n, rhs, c0_, c0_ + k - 1, last_n)

            def emit_consume(b, c, u, states):
                last = [last_chunk(b, n) for n in range(2)]
                for n in range(2):
                    if states[n] == "skip":
                        continue
                    a = clip_a(b, c, n)
                    lo_c, hi_c = n * NHALF + a, (n + 1) * NHALF
                    if states[n] == "clear":
                        r = lrun.get((b, n))
                        if r is not None and r[0] + r[3] == c:
                            c0_, acc, fu, k = r
                            if k == 1:
                                acc = sbp.tile(
                                    [CHUNK, NHALF], F16, tag=f"us{n}"
                                )
                                nc.vector.tensor_add(
                                    acc[:],
                                    fu[:, lo_c:hi_c],
                                    u[:, lo_c:hi_c],
                                )
                            else:
                                nc.vector.tensor_add(
                                    acc[:], acc[:], u[:, lo_c:hi_c]
                                )
                            lrun[(b, n)] = [c0_, acc, fu, k + 1]
                        else:
                            flush_run(b, n, last[n])
                            lrun[(b, n)] = [c, None, u, 1]
                        if c == last[n]:
                            flush_run(b, n, last[n])
                    else:
                        flush_run(b, n, last[n])
                        emit_l(b, n, u[:, lo_c:hi_c], c, c, last[n], a=a)
                    nc.tensor.matmul(
                        o_ps[n][:, a:NHALF],
                        v_t[b][:, c * CHUNK : (c + 1) * CHUNK],
                        u[:, lo_c:hi_c],
                        start=c == 0,
                        stop=c == last[n],
                        skip_group_check=True,
                    )

            def emit_epilogue_half(b, n, terminal):
                half = slice(n * NHALF, (n + 1) * NHALF)
                rl = sbe.tile([CHUNK, NHALF], F32, tag=f"rl{n}")
                nc.vector.reciprocal_approx_fast(rl[:], l_bc[n][:])
                out_sb = sbe.tile([D, NHALF], F16, tag=f"osb{n}")
                nc.vector.tensor_mul(out_sb[:], o_ps[n][:], rl[:])
                nc.sync.dma_start(out_d.ap()[b][:, half], out_sb[:])

            # The final chunk of every sequence always skips half 0 (its last
            # contributing chunk is earlier), so half 0's epilogue can be
            # emitted BEFORE the final chunk's l/PV consumption.
            def flush(pb, pc, pu, pst, terminal):
                if pc == cb[pb] - 1:
                    emit_epilogue_half(pb, 0, terminal=terminal)
                    emit_consume(pb, pc, pu, pst)
                    emit_epilogue_half(pb, 1, terminal=terminal)
                else:
                    emit_consume(pb, pc, pu, pst)

            flat = [(b, c) for b in border for c in range(cb[b])]
            pend = []
            for b, c in flat:
                u, states = emit_score(b, c)
                if len(pend) == 3:
                    pb, pc, pu, pst = pend.pop(0)
                    flush(pb, pc, pu, pst, terminal=False)
                pend.append((b, c, u, states))
            for i, (pb, pc, pu, pst) in enumerate(pend):
                flush(pb, pc, pu, pst, terminal=i == len(pend) - 1)

    nc.compile()
    return nc, L, cb, offs


def _pack_inputs(query, k_cache, v_cache, block_tables, L, cb, offs):
    """Gather the paged cache and pack per-core fp16 shards in device layouts."""
    C = int(offs[-1])
    k_lin = k_cache[block_tables].reshape(B, KV, KVH, D)
    v_lin = v_cache[block_tables].reshape(B, KV, KVH, D)
    kt_all = np.zeros((KVH, D, C * CHUNK), dtype=np.float32)
    v_all = np.zeros((KVH, CHUNK, C * CHUNK), dtype=np.float32)
    for b in range(B):
        Lb, w = int(L[b]), cb[b] * CHUNK
        kk = np.zeros((w, KVH, D), dtype=np.float32)
        kk[:Lb] = k_lin[b, :Lb]
        kt_all[:, :, offs[b] * CHUNK : offs[b] * CHUNK + w] = kk.transpose(
            1, 2, 0
        )
        vv = np.zeros((w, KVH, D), dtype=np.float32)
        vv[:Lb] = v_lin[b, :Lb]
        v_all[:, :, offs[b] * CHUNK : offs[b] * CHUNK + w] = (
            vv.reshape(cb[b], CHUNK, KVH, D)
            .transpose(2, 1, 0, 3)
            .reshape(KVH, CHUNK, w)
        )
    # query [B,Q,H,D] -> [KVH, D, B, Q, G] (t-major, g inner)
    qt_all = (
        query.transpose(2, 3, 0, 1)
        .reshape(KVH, G, D, B, Q)
        .transpose(0, 2, 3, 4, 1)
        .reshape(KVH, D, B * QCOLS)
    )
    kt_all = kt_all.astype(np.float16)
    v_all = v_all.astype(np.float16)
    qt_all = np.ascontiguousarray(qt_all).astype(np.float16)
    return [
        {
            "kt": np.ascontiguousarray(kt_all[h]),
            "v": np.ascontiguousarray(v_all[h]),
            "qt": qt_all[h],
        }
        for h in range(KVH)
    ]


def _unpack_outputs(results):
    """[B,D,QCOLS] fp16 per core (O^T, q=(t,g) on cols) -> [B*Q, H*D] f32."""
    out = np.empty((B * Q, H * D), dtype=np.float32)
    for h, res in enumerate(results):
        o = res["out"].astype(np.float32).reshape(B, D, Q, G)  # [b, d, t, g]
        o = o.transpose(0, 2, 3, 1).reshape(B * Q, G * D)
        out[:, h * G * D : (h + 1) * G * D] = o
    return out


def kernel(query, k_cache, v_cache, block_tables, seq_lens):
    query = np.asarray(query, dtype=np.float32)
    k_cache = np.asarray(k_cache, dtype=np.float32)
    v_cache = np.asarray(v_cache, dtype=np.float32)
    block_tables = np.asarray(block_tables, dtype=np.int64)
    nc, L, cb, offs = _build(np.asarray(seq_lens))
    in_maps = _pack_inputs(query, k_cache, v_cache, block_tables, L, cb, offs)
    res = run_bass_kernel_spmd(nc, in_maps, core_ids=list(range(N_CORES)))
    return _unpack_outputs(res.results)


# revision 42
# speedup vs baseline: 1.0144x; 1.0144x over previous
"""Paged GQA chunked-prefill attention for 8 Trainium2 NeuronCores.

Problem (hardcoded): B=4 seqs x Q=256 new tokens, H=32 query heads, KVH=8 kv
heads (GQA group G=4), D=128 head dim, paged KV cache of 512 blocks x 16
tokens, per-seq lengths in seq_lens (clamped to >= Q), causal masking.

Sharding: tensor-parallel over heads. Core h gets kv head h and query heads
h*4..h*4+3; block_tables/seq_lens are resolved host-side while packing the
shards; the output is all-gathered host-side over the hidden dim.

Per-core device kernel, all matmul operands fp16 (full PE rate, fp32 PSUM
accumulation). For seq b, kv chunk c of 128 positions, q = (t,g) -> 1024
columns in two 512-column halves n; fully-masked column prefixes of boundary
chunks are clipped out of every stage:
  S^T[kv,q] = K_c^T q                 (fp16 matmul into PSUM)
  S^T += causal mask                  (identity-lhsT matmul, boundary band)
  U = exp(SCALE * S^T)                (one ScalarE pass over the active cols)
  lb[128,q] += ones128^T @ U          (all-ones lhsT: the denominator lands
                                       broadcast across all 128 partitions,
                                       so the epilogue needs no cross-
                                       partition moves at all)
  O^T[d,q] += V_c^T @ U               (PSUM accumulation over chunks)
Runs of unmasked chunks accumulate their U on the VectorE (fp16 adds) so one
half-width l matmul covers the whole run. Scores/exp are emitted three chunks
ahead of l/PV consumption (software pipeline) so neither the ScalarE exp nor
the per-sequence epilogues stall the PE. Sequences run longest-first; each
half's epilogue (rl = 1/lb fast-reciprocal straight out of PSUM, then
out = O^T * rl -> fp16, DMA out) is emitted as soon as that half's
accumulation stops, one chunk before the sequence ends for half 0. The PE is
kept warm from t=0 with matmuls on a memset tile (HAM stays at 2.4 GHz) and
the exp table preloads while the input DMAs stream on both HW-DGE rings.
"""
import math

import numpy as np

import concourse.mybir as mybir
import concourse.tile as tile
from concourse import bacc
from concourse.bass_utils import run_bass_kernel_spmd

B, Q, H, D = 4, 256, 32, 128
KVH = 8
G = H // KVH
BLOCK = 16
NB = 128
KV = NB * BLOCK
NUM_BLOCKS = B * NB
SCALE = 1.0 / math.sqrt(D)
N_CORES = 8
CHUNK = 128
QCOLS = G * Q  # 1024 q columns per sequence per core
NHALF = 512

F32 = mybir.dt.float32
F16 = mybir.dt.float16
NEG = -60000.0  # exactly representable in fp16; SCALE*NEG ~ -5300 -> exp = 0


def _plan(seq_lens):
    """Chunk counts, processing order, and tight boundary mask tiles."""
    L = np.maximum(np.asarray(seq_lens, dtype=np.int64), Q)
    cb = [int((int(x) + CHUNK - 1) // CHUNK) for x in L]
    offs = np.concatenate([[0], np.cumsum(cb)]).astype(int)
    border = sorted(range(B), key=lambda b: -cb[b])  # longest first
    # (b, c, n) -> (tmin, tup, mask[128, tup-tmin]); cols t < tmin are fully
    # masked (clipped everywhere), t >= tup fully visible.
    masked = {}
    p = np.arange(CHUNK)
    for b in range(B):
        for c in range(cb[b]):
            for n in range(2):
                lo = int(L[b]) - Q + n * CHUNK  # kv pos of this half's t=0
                if c * CHUNK > lo + CHUNK - 1:
                    continue  # fully masked half
                if c * CHUNK + CHUNK - 1 > lo:
                    tmin = max(0, min(CHUNK, c * CHUNK - lo))
                    tup = max(0, min(CHUNK, c * CHUNK + CHUNK - lo))
                    t = np.arange(tmin, tup)
                    kvpos = c * CHUNK + p
                    m = np.where(
                        kvpos[:, None] > lo + t[None, :], NEG, 0.0
                    ).astype(np.float16)
                    masked[(b, c, n)] = (tmin, tup, m)
    order = sorted(
        masked.keys(), key=lambda k: (border.index(k[0]), k[1], k[2])
    )
    return L, cb, offs, border, masked, order


def _build(seq_lens):
    L, cb, offs, border, masked, order = _plan(seq_lens)
    C = int(offs[-1])
    mask_np = np.concatenate(
        [masked[k][2] for k in order] or [np.zeros((CHUNK, 1), np.float16)],
        axis=1,
    )
    mcols = mask_np.shape[1]
    moff = {}
    acc = 0
    for k in order:
        moff[k] = acc
        acc += masked[k][2].shape[1]
    ident_np = np.eye(CHUNK, dtype=np.float16)
    ones_np = np.ones((CHUNK, CHUNK), dtype=np.float16)

    nc = bacc.Bacc(
        "TRN2", target_bir_lowering=False, debug=False, num_devices=N_CORES
    )
    kt_d = nc.dram_tensor("kt", [D, C * CHUNK], F16, kind="ExternalInput")
    v_d = nc.dram_tensor("v", [CHUNK, C * CHUNK], F16, kind="ExternalInput")
    qt_d = nc.dram_tensor("qt", [D, B * QCOLS], F16, kind="ExternalInput")
    out_d = nc.dram_tensor("out", [B, D, QCOLS], F16, kind="ExternalOutput")
    mask_d = nc.inline_tensor(mask_np, name="mask_const")
    ident_d = nc.inline_tensor(ident_np, name="ident_const")
    ones_d = nc.inline_tensor(ones_np, name="ones_const")

    exp = mybir.ActivationFunctionType.Exp

    def half_lo(b, n):
        return int(L[b]) - Q + n * CHUNK

    def half_state(b, c, n):
        if c * CHUNK > half_lo(b, n) + CHUNK - 1:
            return "skip"
        if (b, c, n) in masked:
            return "mask"
        return "clear"

    def last_chunk(b, n):
        return min(cb[b] - 1, (half_lo(b, n) + CHUNK - 1) // CHUNK)

    def clip_a(b, c, n):
        """First active column (of 512) for this chunk-half."""
        if (b, c, n) in masked:
            return G * masked[(b, c, n)][0]
        return 0

    with tile.TileContext(nc) as tc:
        with (
            tc.tile_pool(name="sbin", bufs=1) as sbin,
            tc.tile_pool(name="sbu", bufs=8) as sbu,
            tc.tile_pool(name="sbp", bufs=2) as sbp,
            tc.tile_pool(name="sbe", bufs=3) as sbe,
            tc.tile_pool(name="ps_s", bufs=2, space="PSUM") as ps_s,
            tc.tile_pool(name="ps_o", bufs=1, space="PSUM") as ps_o,
            tc.tile_pool(name="ps_l", bufs=1, space="PSUM") as ps_l,
        ):
            # Warm the PE + load the exp table before any DMA lands: matmuls
            # on a memset tile keep the HAM activity window busy so the clock
            # is at 2.4 GHz when real data arrives.
            warm_w = sbin.tile([CHUNK, CHUNK], F16, tag="warmw")
            nc.vector.memset(warm_w[:], 0.0)
            dummy = sbe.tile([CHUNK, 1], F32, tag="dummy")
            nc.scalar.activation(dummy[:], warm_w[:, 0:1], exp)
            # l broadcast accumulators (one tile per half so the early
            # half-0 epilogue never false-shares with half-1 matmuls); the
            # first tile doubles as the warmup target -- the first real
            # l matmul (start=True) resets it.
            l_bc0 = ps_l.tile([CHUNK, NHALF], F32, tag="lbc0")
            l_bc1 = ps_l.tile([CHUNK, NHALF], F32, tag="lbc1")
            l_bc = (l_bc0, l_bc1)
            for _ in range(40):
                nc.tensor.matmul(
                    l_bc0[:, 0:CHUNK], warm_w[:], warm_w[:],
                    start=True, stop=True, skip_group_check=True,
                )

            # Input DMAs in processing order (longest seq first). First-seq
            # loads split across the two HW-DGE rings (SP + ACT) so their
            # issue costs don't serialize; head chunks land first so compute
            # can start while the rest of the long sequence streams.
            b0 = border[0]
            kt_t = [None] * B
            v_t = [None] * B
            qt_t = [None] * B
            w0 = cb[b0] * CHUNK
            o0_ = offs[b0] * CHUNK
            head0 = min(3 * CHUNK, w0)
            qt0 = sbin.tile([D, QCOLS], F16, tag=f"qt{b0}")
            nc.sync.dma_start(
                qt0[:, 0:NHALF], qt_d.ap()[:, b0 * QCOLS : b0 * QCOLS + NHALF]
            )
            nc.scalar.dma_start(
                qt0[:, NHALF:QCOLS],
                qt_d.ap()[:, b0 * QCOLS + NHALF : (b0 + 1) * QCOLS],
            )
            kt0 = sbin.tile([D, w0], F16, tag=f"kt{b0}")
            nc.sync.dma_start(kt0[:, 0:head0], kt_d.ap()[:, o0_ : o0_ + head0])
            v0 = sbin.tile([CHUNK, w0], F16, tag=f"v{b0}")
            nc.scalar.dma_start(v0[:, 0:head0], v_d.ap()[:, o0_ : o0_ + head0])
            ones_t = sbin.tile([CHUNK, CHUNK], F16, tag="ones")
            nc.sync.dma_start(ones_t[:], ones_d.ap())
            identr = sbin.tile([CHUNK, CHUNK], F16, tag="identr")
            nc.scalar.dma_start(identr[:], ident_d.ap())
            if head0 < w0:
                mid0 = (head0 + w0 + CHUNK) // (2 * CHUNK) * CHUNK
                nc.sync.dma_start(
                    kt0[:, head0:mid0], kt_d.ap()[:, o0_ + head0 : o0_ + mid0]
                )
                nc.scalar.dma_start(
                    v0[:, head0:mid0], v_d.ap()[:, o0_ + head0 : o0_ + mid0]
                )
                if mid0 < w0:
                    nc.sync.dma_start(
                        kt0[:, mid0:w0], kt_d.ap()[:, o0_ + mid0 : o0_ + w0]
                    )
                    nc.scalar.dma_start(
                        v0[:, mid0:w0], v_d.ap()[:, o0_ + mid0 : o0_ + w0]
                    )
            kt_t[b0], v_t[b0], qt_t[b0] = kt0, v0, qt0

            masks = sbin.tile([CHUNK, mcols], F16, tag="masks")
            cut = sum(
                masked[k][2].shape[1]
                for k in order
                if border.index(k[0]) <= 1
            )
            cut = max(1, min(cut, mcols))
            nc.sync.dma_start(masks[:, 0:cut], mask_d.ap()[:, 0:cut])

            for b in border[1:]:
                w = cb[b] * CHUNK
                o0 = offs[b] * CHUNK
                qt = sbin.tile([D, QCOLS], F16, tag=f"qt{b}")
                nc.sync.dma_start(qt[:], qt_d.ap()[:, b * QCOLS : (b + 1) * QCOLS])
                kt = sbin.tile([D, w], F16, tag=f"kt{b}")
                vt = sbin.tile([CHUNK, w], F16, tag=f"v{b}")
                head = min(4 * CHUNK, w)
                nc.sync.dma_start(kt[:, 0:head], kt_d.ap()[:, o0 : o0 + head])
                nc.sync.dma_start(vt[:, 0:head], v_d.ap()[:, o0 : o0 + head])
                if head < w:
                    nc.sync.dma_start(
                        kt[:, head:w], kt_d.ap()[:, o0 + head : o0 + w]
                    )
                    nc.sync.dma_start(
                        vt[:, head:w], v_d.ap()[:, o0 + head : o0 + w]
                    )
                kt_t[b], v_t[b], qt_t[b] = kt, vt, qt
            if cut < mcols:
                nc.sync.dma_start(
                    masks[:, cut:mcols], mask_d.ap()[:, cut:mcols]
                )

            o_ps0 = ps_o.tile([D, NHALF], F32, tag="o0")
            o_ps1 = ps_o.tile([D, NHALF], F32, tag="o1")
            o_ps = (o_ps0, o_ps1)

            def emit_score(b, c):
                states = [half_state(b, c, n) for n in range(2)]
                s_ps = ps_s.tile([CHUNK, QCOLS], F32, tag="s")
                for n in range(2):
                    if states[n] == "skip":
                        continue
                    h0 = n * NHALF
                    if states[n] == "clear":
                        nc.tensor.matmul(
                            s_ps[:, h0 : h0 + NHALF],
                            kt_t[b][:, c * CHUNK : (c + 1) * CHUNK],
                            qt_t[b][:, h0 : h0 + NHALF],
                            start=True,
                            stop=True,
                        )
                        continue
                    tmin, tup, _ = masked[(b, c, n)]
                    a, u_c = G * tmin, G * tup
                    nc.tensor.matmul(
                        s_ps[:, h0 + a : h0 + u_c],
                        kt_t[b][:, c * CHUNK : (c + 1) * CHUNK],
                        qt_t[b][:, h0 + a : h0 + u_c],
                        start=True,
                        stop=False,
                    )
                    mo = moff[(b, c, n)]
                    mb = (
                        masks[:, mo : mo + (tup - tmin)]
                        .unsqueeze(2)
                        .broadcast_to([CHUNK, tup - tmin, G])
                    )
                    nc.tensor.matmul(
                        s_ps[:, h0 + a : h0 + u_c],
                        identr[:],
                        mb,
                        start=False,
                        stop=True,
                    )
                    if u_c < NHALF:
                        nc.tensor.matmul(
                            s_ps[:, h0 + u_c : h0 + NHALF],
                            kt_t[b][:, c * CHUNK : (c + 1) * CHUNK],
                            qt_t[b][:, h0 + u_c : h0 + NHALF],
                            start=True,
                            stop=True,
                        )
                act0 = (
                    clip_a(b, c, 0)
                    if states[0] != "skip"
                    else NHALF + clip_a(b, c, 1)
                )
                u = sbu.tile([CHUNK, QCOLS], F16, tag="u")
                nc.scalar.activation(
                    u[:, act0:QCOLS], s_ps[:, act0:QCOLS], exp, scale=SCALE
                )
                return u, states

            # (b, n) -> [c_start, acc_ap_or_None, first_u, count]: running
            # U-sum of a clear-chunk run; one l matmul per run.
            lrun = {}

            def emit_l(b, n, rhs_ap, c0_, c1_, last_n, a=0):
                nc.tensor.matmul(
                    l_bc[n][:, a:NHALF],
                    ones_t[:],
                    rhs_ap,
                    start=c0_ == 0,
                    stop=c1_ == last_n,
                    skip_group_check=True,
                )

            def flush_run(b, n, last_n):
                r = lrun.pop((b, n), None)
                if r is None:
                    return
                c0_, acc, fu, k = r
                rhs = fu[:, n * NHALF : (n + 1) * NHALF] if k == 1 else acc[:]
                emit_l(b, n, rhs, c0_, c0_ + k - 1, last_n)

            def emit_consume(b, c, u, states):
                last = [last_chunk(b, n) for n in range(2)]
                for n in range(2):
                    if states[n] == "skip":
                        continue
                    a = clip_a(b, c, n)
                    lo_c, hi_c = n * NHALF + a, (n + 1) * NHALF
                    if states[n] == "clear":
                        r = lrun.get((b, n))
                        if r is not None and r[0] + r[3] == c:
                            c0_, acc, fu, k = r
                            if k == 1:
                                acc = sbp.tile(
                                    [CHUNK, NHALF], F16, tag=f"us{n}"
                                )
                                nc.vector.tensor_add(
                                    acc[:],
                                    fu[:, lo_c:hi_c],
                                    u[:, lo_c:hi_c],
                                )
                            else:
                                nc.vector.tensor_add(
                                    acc[:], acc[:], u[:, lo_c:hi_c]
                                )
                            lrun[(b, n)] = [c0_, acc, fu, k + 1]
                        else:
                            flush_run(b, n, last[n])
                            lrun[(b, n)] = [c, None, u, 1]
                        if c == last[n]:
                            flush_run(b, n, last[n])
                    else:
                        flush_run(b, n, last[n])
                        emit_l(b, n, u[:, lo_c:hi_c], c, c, last[n], a=a)
                    nc.tensor.matmul(
                        o_ps[n][:, a:NHALF],
                        v_t[b][:, c * CHUNK : (c + 1) * CHUNK],
                        u[:, lo_c:hi_c],
                        start=c == 0,
                        stop=c == last[n],
                        skip_group_check=True,
                    )

            def emit_epilogue_half(b, n, terminal):
                half = slice(n * NHALF, (n + 1) * NHALF)
                rl = sbe.tile([CHUNK, NHALF], F32, tag=f"rl{n}")
                nc.vector.reciprocal_approx_fast(rl[:], l_bc[n][:])
                out_sb = sbe.tile([D, NHALF], F16, tag=f"osb{n}")
                nc.vector.tensor_mul(out_sb[:], o_ps[n][:], rl[:])
                nc.sync.dma_start(out_d.ap()[b][:, half], out_sb[:])

            # The final chunk of every sequence always skips half 0 (its last
            # contributing chunk is earlier), so half 0's epilogue can be
            # emitted BEFORE the final chunk's l/PV consumption.
            def flush(pb, pc, pu, pst, terminal):
                if pc == cb[pb] - 1:
                    emit_epilogue_half(pb, 0, terminal=terminal)
                    emit_consume(pb, pc, pu, pst)
                    emit_epilogue_half(pb, 1, terminal=terminal)
                else:
                    emit_consume(pb, pc, pu, pst)

            flat = [(b, c) for b in border for c in range(cb[b])]
            pend = []
            for b, c in flat:
                u, states = emit_score(b, c)
                if len(pend) == 3:
                    pb, pc, pu, pst = pend.pop(0)
                    flush(pb, pc, pu, pst, terminal=False)
                pend.append((b, c, u, states))
            for i, (pb, pc, pu, pst) in enumerate(pend):
                flush(pb, pc, pu, pst, terminal=i == len(pend) - 1)

    nc.compile()
    return nc, L, cb, offs


def _pack_inputs(query, k_cache, v_cache, block_tables, L, cb, offs):
    """Gather the paged cache and pack per-core fp16 shards in device layouts."""
    C = int(offs[-1])
    k_lin = k_cache[block_tables].reshape(B, KV, KVH, D)
    v_lin = v_cache[block_tables].reshape(B, KV, KVH, D)
    kt_all = np.zeros((KVH, D, C * CHUNK), dtype=np.float32)
    v_all = np.zeros((KVH, CHUNK, C * CHUNK), dtype=np.float32)
    for b in range(B):
        Lb, w = int(L[b]), cb[b] * CHUNK
        kk = np.zeros((w, KVH, D), dtype=np.float32)
        kk[:Lb] = k_lin[b, :Lb]
        kt_all[:, :, offs[b] * CHUNK : offs[b] * CHUNK + w] = kk.transpose(
            1, 2, 0
        )
        vv = np.zeros((w, KVH, D), dtype=np.float32)
        vv[:Lb] = v_lin[b, :Lb]
        v_all[:, :, offs[b] * CHUNK : offs[b] * CHUNK + w] = (
            vv.reshape(cb[b], CHUNK, KVH, D)
            .transpose(2, 1, 0, 3)
            .reshape(KVH, CHUNK, w)
        )
    # query [B,Q,H,D] -> [KVH, D, B, Q, G] (t-major, g inner)
    qt_all = (
        query.transpose(2, 3, 0, 1)
        .reshape(KVH, G, D, B, Q)
        .transpose(0, 2, 3, 4, 1)
        .reshape(KVH, D, B * QCOLS)
    )
    kt_all = kt_all.astype(np.float16)
    v_all = v_all.astype(np.float16)
    qt_all = np.ascontiguousarray(qt_all).astype(np.float16)
    return [
        {
            "kt": np.ascontiguousarray(kt_all[h]),
            "v": np.ascontiguousarray(v_all[h]),
            "qt": qt_all[h],
        }
        for h in range(KVH)
    ]


def _unpack_outputs(results):
    """[B,D,QCOLS] fp16 per core (O^T, q=(t,g) on cols) -> [B*Q, H*D] f32."""
    out = np.empty((B * Q, H * D), dtype=np.float32)
    for h, res in enumerate(results):
        o = res["out"].astype(np.float32).reshape(B, D, Q, G)  # [b, d, t, g]
        o = o.transpose(0, 2, 3, 1).reshape(B * Q, G * D)
        out[:, h * G * D : (h + 1) * G * D] = o
    return out


def kernel(query, k_cache, v_cache, block_tables, seq_lens):
    query = np.asarray(query, dtype=np.float32)
    k_cache = np.asarray(k_cache, dtype=np.float32)
    v_cache = np.asarray(v_cache, dtype=np.float32)
    block_tables = np.asarray(block_tables, dtype=np.int64)
    nc, L, cb, offs = _build(np.asarray(seq_lens))
    in_maps = _pack_inputs(query, k_cache, v_cache, block_tables, L, cb, offs)
    res = run_bass_kernel_spmd(nc, in_maps, core_ids=list(range(N_CORES)))
    return _unpack_outputs(res.results)
